# revision 1
# baseline (speedup 1.0000x reference)
"""BuildCostVolume kernel for 8 Trainium2 NeuronCores.

Decomposition: the 9 strided dilated convs (disparities d=-4..4) reduce to
729 taps (d,u,v): cost[b,co,d,h,w] = sum_{ci,u,v} Wd[d][co,ci,u,v] *
X[b,ci,u,v, h+d(4-u), w+d(4-v)] where X is the view-decomposed light field
(X[b,ci,u,v,h,w] = x[b,ci,9h+u,9w+v], zero outside) and Wd flips (u,v) for
d>0.  Each tap is a K=64(ci) x M=64(co) matmul over spatial positions.

Sharding: core = (batch b, h-half).  Each core holds all 81 views' h-windows
(zero-padded to uniform per-u heights) resident in SBUF as 41 view-pair
tiles ([128, R*48] bf16: two views stacked on partition halves).  Taps run
as 4-way concurrent matmuls via tile_position (2 row-groups x 2 col-groups),
accumulating per (d, 8-row subchunk) into PSUM tiles keyed (sub, rh) with
the col-group picking the partition half.  Evac: the Activation engine
copies one psum quadrant-partial while DVE does the 3 accumulating adds
(cross-partition reads are only legal from PSUM; one PSUM operand per op)
— moving the copy off DVE took the measured sweep from ~136us to ~112us.
Input load ships only the valid rows of each view block (79% of bytes; the
load is HBM-bound across the 8 cores) and branches on core id (tc.If on
nc.partition_id(), which loads the condition on ALL engines — an SP-only
register deadlocks the DVE memsets) to zero the class-dependent pad rows
on the otherwise-idle DVE while the DMAs stream.  Weights are shipped once
as the 81 pre-transposed taps [ci, co] (10KB/partition, resident in SBUF;
lhsT APs index them by (kh,kw) per (d,tap)) instead of a per-d wpack —
every disparity reuses the same 81 slices, so the old layout shipped the
weight bytes 4.5x over and spent 9 DMA sync points per sweep.

The (4,4) singleton view (the only unpaired one) is duplicated onto its
tile's empty half and its matmul split into two half-width MMs, one per
row-half on the per-d lightest quadrant — without this the 41-vs-40 tap
split between row halves inflates the 4-quadrant wall by ~3.9% (~3us).

Measured dead ends (see _transcript/memory): fp8e4 DoubleRow is blocked by
the s3d3_mm_valid_dst_partition ISA check at col position 64 (DoubleRow's
128-wide stationary uses all PE columns -> only 2-way row tiling), and the
3-term error compensation needed for rel_err<2e-2 makes it net slower than
bf16.  A ch-fixed group restructure (2 psum partials/group, 4-group sets)
measured ~163us/sweep — worse than this layout.
"""

import numpy as np
import ml_dtypes

A = 9           # angular resolution
H = 48          # spatial h/w per view
C = 64          # channels (ci = co = 64)
B = 4           # batch
ND = 9          # disparities -4..4
HH = 24         # h rows per core (half)
SUB = 8         # output h rows per psum accumulation group
NSUB = HH // SUB
NSLOT = 41      # weight slots per row-half per d
N_CORES = 8

BF16 = ml_dtypes.bfloat16


def _geometry():
    """Static tap/tile geometry shared by host packing and device program."""
    pairs = []            # (viewA, viewB-or-None, R)
    # seed tiles first: their 4 views are the full-width (v=4) taps that
    # carry start=True per tile position, so their DMAs must land first.
    pairs.append(((0, 4), (8, 4), HH + 8 * 4))
    pairs.append(((1, 4), (7, 4), HH + 8 * 3))
    for v in range(A):
        for u in range(4):
            if v == 4 and u in (0, 1):
                continue
            pairs.append(((u, v), (8 - u, v), HH + 8 * (4 - u)))
    for k in range(4):
        pairs.append(((4, k), (4, k + 5), HH))
    pairs.append(((4, 4), None, HH))

    view_loc = {}
    offs = []
    off = 0
    for j, (va, vb, R) in enumerate(pairs):
        view_loc[va] = (j, 0)
        if vb is not None:
            view_loc[vb] = (j, 1)
        offs.append(off)
        off += R * H
    F = off

    # one tap order for every d: tile order (= DMA arrival order), seeds
    # first covering the 4 (rh, ch) positions, then ch alternating per rh
    # so consecutive taps hit different tile positions.
    taps = []
    ch_ctr = {0: 0, 1: 0}
    slot_ctr = {0: 0, 1: 0}
    for j, (va, vb, R) in enumerate(pairs):
        for half, view in ((0, va), (1, vb)):
            if view is None:
                continue
            u, v = view
            rh = half
            ch = ch_ctr[rh] % 2
            ch_ctr[rh] += 1
            s = slot_ctr[rh]
            slot_ctr[rh] += 1
            taps.append((u, v, rh, ch, s))
    assert len(taps) == 81
    # seeds sanity: first 4 taps cover all 4 positions with v=4 views
    seed_pos = {(rh, ch) for (u, v, rh, ch, s) in taps[:4]}
    assert len(seed_pos) == 4 and all(v == 4 for (u, v, _, _, _) in taps[:4])

    return pairs, view_loc, offs, F, taps


_PAIRS, _VIEW_LOC, _OFFS, _F, _TAPS1 = _geometry()
_TAPS = [_TAPS1] * ND  # same order for every d (kept for prepare_inputs)
_NC_CACHE = {}


def _build_nc(repeat=1):
    import os
    import concourse.bacc as bacc
    import concourse.mybir as mybir
    import concourse.tile as tile

    variant = os.environ.get("KVARIANT", "full")  # timing experiments only

    nc = bacc.Bacc(None, target_bir_lowering=False)
    xwin_d = nc.dram_tensor("xwin", [128, _F], mybir.dt.bfloat16,
                            kind="ExternalInput")
    # 81 pre-transposed W taps [ci, co], duplicated on both partition
    # halves: every (d, tap) weight block is one of these slices, so no
    # per-d wpack staging (saves 5.3MB/core of the HBM-bound input and 9
    # DMA sync points per sweep).
    wt_d = nc.dram_tensor("wt", [128, A * A * C], mybir.dt.bfloat16,
                          kind="ExternalInput")
    out_d = nc.dram_tensor("out", [C, ND * NSUB * SUB * H], mybir.dt.float32,
                           kind="ExternalOutput")

    with tile.TileContext(nc) as tc:
        with tc.tile_pool(name="xw", bufs=1) as xpool, \
             tc.tile_pool(name="wp", bufs=1) as wpool, \
             tc.tile_pool(name="ps", bufs=2, space="PSUM") as ppool, \
             tc.tile_pool(name="ob", bufs=4) as opool:

            # resident weight taps; DMA first so the seed MMs aren't blocked
            wtr = wpool.tile([128, A * A * C], mybir.dt.bfloat16, tag="wtr")
            nc.sync.dma_start(out=wtr[:], in_=wt_d[:])

            xtiles = []
            xviews = []
            for j, (va, vb, R) in enumerate(_PAIRS):
                t = xpool.tile([128, R * H], mybir.dt.bfloat16, tag=f"x{j}")
                xtiles.append(t)
                # h-major [p, r, c]: contiguous 48-elem inner runs stream at
                # full PE rate (short strided inner runs are ~2.5x slower)
                xviews.append(t[:].rearrange("p (r c) -> p r c", r=R, c=H))

            def load_x():
                # Valid-rows-only input: each view block [R=24+8au, 48] has
                # 4au zero rows (top for h-half 0 cores, bottom for h-half 1)
                # that the host packs but we never ship — DVE memsets them
                # while the DMAs stream the remaining 79%.  Both views of a
                # pair share au, so pad ranges are tile-uniform.  The pad
                # side depends on the core's h-half -> branch on core id.
                pid = nc.partition_id()
                for cls in (0, 1):
                    with tc.If(pid % 2 == cls):
                        for j, (va, vb, R) in enumerate(_PAIRS):
                            pad = ((R - HH) // 8) * 4  # 4*au
                            s, e = (pad, R) if cls == 0 else (0, R - pad)
                            if pad:
                                z0, z1 = ((0, pad) if cls == 0
                                          else (R - pad, R))
                                nc.vector.memset(
                                    xtiles[j][:, z0 * H:z1 * H], 0.0)
                            nc.sync.dma_start(
                                out=xtiles[j][:, s * H:e * H],
                                in_=xwin_d[:, _OFFS[j] + s * H:
                                           _OFFS[j] + e * H])

            taps = _TAPS1
            last_pos = {}
            for i, (u, v, rh, ch, s) in enumerate(taps):
                last_pos[(rh, ch)] = i

            def evac(di, sub, ptA, ptB):
                ot = opool.tile([64, SUB * H], mybir.dt.float32, tag="ot")
                if variant == "dve4":
                    # walrus: only one non-scalar input may read PSUM per op
                    nc.vector.tensor_copy(ot[:], ptA[0:64, :])
                    nc.vector.tensor_add(ot[:], ot[:], ptA[64:128, :])
                    nc.vector.tensor_add(ot[:], ot[:], ptB[0:64, :])
                    nc.vector.tensor_add(ot[:], ot[:], ptB[64:128, :])
                elif variant == "evac1":
                    nc.vector.tensor_copy(ot[:], ptA[0:64, :])
                else:
                    # Act engine does the copy (psum read is partition-
                    # aligned); DVE does the 3 accumulating adds.  Cross-
                    # partition reads are allowed only from PSUM, and only
                    # one PSUM operand per op (walrus rules).
                    nc.scalar.activation(ot[:], ptA[0:64, :],
                                         mybir.ActivationFunctionType.Copy)
                    nc.vector.tensor_add(ot[:], ot[:], ptA[64:128, :])
                    nc.vector.tensor_add(ot[:], ot[:], ptB[0:64, :])
                    nc.vector.tensor_add(ot[:], ot[:], ptB[64:128, :])
                seg = (di * NSUB + sub) * SUB * H
                nc.sync.dma_start(out=out_d[:, seg:seg + SUB * H], in_=ot[:])

            def mm(d, sub, tap, banks, started):
                (u, v, rh, ch, s) = tap
                j, half = _VIEW_LOC[(u, v)]
                au = abs(4 - u)
                row0 = sub * SUB + d * (4 - u) + 4 * au
                sv = d * (4 - v)
                if variant == "fullw":
                    # timing-only: full-width windows (wrong numerics) to
                    # measure the cost of short-inner-run clipped APs
                    sv = 0
                wlo = max(0, -sv)
                whi = min(H, H - sv)
                rhs = xviews[j][rh * 64:(rh + 1) * 64,
                                row0:row0 + SUB,
                                wlo + sv:whi + sv]
                kh, kw = (u, v) if d <= 0 else (8 - u, 8 - v)
                kidx = kh * A + kw
                lhsT = wtr[rh * 64:(rh + 1) * 64,
                           kidx * C:(kidx + 1) * C]
                pt = banks[rh]
                if wlo == 0 and whi == H:
                    outap = pt[ch * 64:(ch + 1) * 64, :]
                else:
                    # 3D psum out AP (h stride 48, contiguous w run): HW
                    # handles it; CoreSim needs the assert patch in test.py.
                    ptv = pt[:].rearrange("p (r c) -> p r c", r=SUB, c=H)
                    outap = ptv[ch * 64:(ch + 1) * 64, 0:SUB, wlo:whi]
                key = (sub, rh, ch)
                nc.tensor.matmul(
                    outap, lhsT, rhs,
                    start=(key not in started),
                    stop=False,
                    tile_position=(rh * 64, ch * 64),
                    skip_group_check=True,
                )
                started.add(key)

            # per-d lightest ch per rh, for placing the (4,4) half-MMs
            ch44 = {}
            for dd in range(-4, 5):
                base = {(r, c): 0 for r in (0, 1) for c in (0, 1)}
                for (u, v, rh, ch, s) in taps:
                    if (u, v) == (4, 4):
                        continue
                    base[(rh, ch)] += H - abs(dd * (4 - v))
                for r in (0, 1):
                    ch44[(dd, r)] = 0 if base[(r, 0)] <= base[(r, 1)] else 1

            def mm44(d, sub, banks, started):
                # (4,4) singleton: no shift/clip for any d; split into two
                # half-width MMs, one per row-half (data duplicated on both
                # tile halves), each on that rh's lightest quadrant.
                j, _ = _VIEW_LOC[(4, 4)]
                kidx = 4 * A + 4
                row0 = sub * SUB
                for rh, c0, c1 in ((0, 0, H // 2), (1, H // 2, H)):
                    ch = ch44[(d, rh)]
                    rhs = xviews[j][rh * 64:(rh + 1) * 64,
                                    row0:row0 + SUB, c0:c1]
                    lhsT = wtr[rh * 64:(rh + 1) * 64,
                               kidx * C:(kidx + 1) * C]
                    pt = banks[rh]
                    ptv = pt[:].rearrange("p (r c) -> p r c", r=SUB, c=H)
                    outap = ptv[ch * 64:(ch + 1) * 64, 0:SUB, c0:c1]
                    key = (sub, rh, ch)
                    nc.tensor.matmul(
                        outap, lhsT, rhs,
                        start=(key not in started), stop=False,
                        tile_position=(rh * 64, ch * 64),
                        skip_group_check=True,
                    )
                    started.add(key)

            def alloc_banks():
                return {
                    (0, 0): ppool.tile([128, SUB * H], mybir.dt.float32,
                                       name="bA0", tag="ptA0"),
                    (0, 1): ppool.tile([128, SUB * H], mybir.dt.float32,
                                       name="bB0", tag="ptB0"),
                    (1, 0): ppool.tile([128, SUB * H], mybir.dt.float32,
                                       name="bA1", tag="ptA1"),
                    (1, 1): ppool.tile([128, SUB * H], mybir.dt.float32,
                                       name="bB1", tag="ptB1"),
                }

            def pass1(d, bank, started):
                for tap in taps:
                    if (tap[0], tap[1]) == (4, 4):
                        mm44(d, 0, (bank[(0, 0)], bank[(0, 1)]), started)
                        mm44(d, 1, (bank[(1, 0)], bank[(1, 1)]), started)
                        continue
                    mm(d, 0, tap, (bank[(0, 0)], bank[(0, 1)]), started)
                    mm(d, 1, tap, (bank[(1, 0)], bank[(1, 1)]), started)

            def emit_sweep():
                # concurrent tile-position matmuls must write DISJOINT psum
                # regions: rh0 -> bank A, rh1 -> bank B, ch picks the
                # partition half.  The first TWO disparities' pass-1 walks
                # are interleaved per tap so ~16us of compute (instead of
                # ~8) overlaps the HBM-bound input stream; the tag rings
                # (bufs=2) hold both generations.  LDWEIGHTS of same-
                # weight MM pairs are removed by _dedup_ldweights.
                bankP = alloc_banks()
                bankQ = alloc_banks()
                stP, stQ = set(), set()
                for tap in taps:
                    if (tap[0], tap[1]) == (4, 4):
                        mm44(-4, 0, (bankP[(0, 0)], bankP[(0, 1)]), stP)
                        mm44(-4, 1, (bankP[(1, 0)], bankP[(1, 1)]), stP)
                        mm44(-3, 0, (bankQ[(0, 0)], bankQ[(0, 1)]), stQ)
                        mm44(-3, 1, (bankQ[(1, 0)], bankQ[(1, 1)]), stQ)
                        continue
                    mm(-4, 0, tap, (bankP[(0, 0)], bankP[(0, 1)]), stP)
                    mm(-4, 1, tap, (bankP[(1, 0)], bankP[(1, 1)]), stP)
                    mm(-3, 0, tap, (bankQ[(0, 0)], bankQ[(0, 1)]), stQ)
                    mm(-3, 1, tap, (bankQ[(1, 0)], bankQ[(1, 1)]), stQ)
                for dd, bk in ((-4, bankP), (-3, bankQ)):
                    evac(dd + 4, 0, bk[(0, 0)], bk[(0, 1)])
                    evac(dd + 4, 1, bk[(1, 0)], bk[(1, 1)])

                for di in range(ND):
                    d = di - 4
                    if d > -3:
                        # pass 1: subs 0,1 (two MMs per tap, shared
                        # weights); d=-4,-3 already ran theirs above
                        bank = alloc_banks()
                        started = set()
                        pass1(d, bank, started)
                        evac(di, 0, bank[(0, 0)], bank[(0, 1)])
                        evac(di, 1, bank[(1, 0)], bank[(1, 1)])
                    # pass 2: sub 2
                    pA = ppool.tile([128, SUB * H], mybir.dt.float32,
                                    name="pA", tag="ptA0")
                    pB = ppool.tile([128, SUB * H], mybir.dt.float32,
                                    name="pB", tag="ptB0")
                    started2 = set()
                    for tap in taps:
                        if (tap[0], tap[1]) == (4, 4):
                            mm44(d, 2, (pA, pB), started2)
                            continue
                        mm(d, 2, tap, (pA, pB), started2)
                    evac(di, 2, pA, pB)

            if repeat == 1:
                load_x()
                emit_sweep()
            elif variant == "loopall":
                # timing-only: input DMA repeats with the sweep, so the
                # loop slope approximates a full single-shot exec
                with tc.For_i(0, repeat, 1):
                    load_x()
                    emit_sweep()
            else:
                # timing-only: repeat the full sweep in a hardware loop
                load_x()
                with tc.For_i(0, repeat, 1):
                    emit_sweep()

    _dedup_ldweights(nc)
    nc.finalize()
    return nc


def _dedup_ldweights(nc):
    """Remove InstLdweights that reload the stationary operand already
    resident at the same tile position (sub-pair MMs share weights).  The
    PE keeps independent stationary sets per (row, col) tile group, and
    only an LDW targeting the same position clobbers one."""
    removed = kept = 0
    for bb in nc.m.functions[0].blocks:
        last = {}
        to_remove = []
        for ins in bb.instructions:
            if not str(ins.engine).endswith("PE"):
                continue
            tn = type(ins).__name__
            if tn == "InstLdweights":
                si = ins.sync_info
                has_sync = si is not None and (si.on_wait or si.on_update)
                sig = (str(ins.ins[0]), str(getattr(ins, "tile_position", None)),
                       str(getattr(ins, "perf_mode", None)))
                pos = str(getattr(ins, "tile_position", None))
                if not has_sync and last.get(pos) == sig:
                    to_remove.append(ins)
                    removed += 1
                else:
                    last[pos] = sig
                    kept += 1
            elif tn == "InstMatmult":
                continue
            else:
                last.clear()
        for ins in to_remove:
            bb.instructions.remove(ins)
    if removed:
        import logging
        logging.getLogger(__name__).info(
            "dedup_ldweights: removed %d, kept %d", removed, kept)


def get_nc(repeat=1):
    import os
    key = ("nc", repeat, os.environ.get("KVARIANT", "full"))
    if key not in _NC_CACHE:
        _NC_CACHE[key] = _build_nc(repeat)
    return _NC_CACHE[key]


def prepare_inputs(x, W):
    """Host-side packing: per-core xwin [128,F] bf16 + shared wpack."""
    x = np.asarray(x, dtype=np.float32)
    W = np.asarray(W, dtype=np.float32)
    # X5[b,u,v,ci,h,w]
    X5 = np.ascontiguousarray(
        x.reshape(B, C, H, A, H, A).transpose(0, 3, 5, 1, 2, 4)
    ).astype(BF16)

    xwins = []
    for core in range(N_CORES):
        b, hh = divmod(core, 2)
        h0 = hh * HH
        xw = np.zeros((128, _F), dtype=BF16)
        for j, (va, vb, R) in enumerate(_PAIRS):
            # the (4,4) singleton is duplicated onto the (otherwise empty)
            # second half of its tile so its matmul can be split across
            # both row-halves for quadrant load balance (same DMA bytes —
            # that half shipped zeros before).
            for half, view in ((0, va), (1, vb if vb is not None else va)):
                u, v = view
                lo = h0 - 4 * abs(4 - u)
                vs = max(0, lo)
                ve = min(H, lo + R)
                blk = X5[b, u, v, :, vs:ve, :]  # [64, ve-vs, 48]
                dst = xw[half * 64:(half + 1) * 64,
                         _OFFS[j]:_OFFS[j] + R * H].reshape(64, R, H)
                dst[:, vs - lo:ve - lo, :] = blk
        xwins.append(xw)

    # wt[ci + 64*half, (kh*9+kw)*64 + co] = W[co, ci, kh, kw], both halves
    wt1 = np.ascontiguousarray(
        W.transpose(1, 2, 3, 0).reshape(C, A * A * C)).astype(BF16)
    wtrans = np.concatenate([wt1, wt1], axis=0)
    return xwins, wtrans


def assemble_output(results):
    """results: list of 8 dicts with 'out' [64, ND*NSUB*SUB*H] fp32."""
    full = np.empty((B, C, ND, H, H), dtype=np.float32)
    for core in range(N_CORES):
        b, hh = divmod(core, 2)
        oc = np.asarray(results[core]["out"]).reshape(C, ND, HH, H)
        full[b, :, :, hh * HH:(hh + 1) * HH, :] = oc
    return full


def make_in_maps(x, W):
    xwins, wtrans = prepare_inputs(x, W)
    return [{"xwin": xwins[c], "wt": wtrans} for c in range(N_CORES)]


def kernel(x, W):
    from concourse.bass_utils import run_bass_kernel_spmd

    nc = get_nc()
    in_maps = make_in_maps(x, W)
    res = run_bass_kernel_spmd(nc, in_maps, core_ids=list(range(N_CORES)))
    return assemble_output(res.results)



# revision 2
# speedup vs baseline: 1.0339x; 1.0339x over previous
"""BuildCostVolume kernel for 8 Trainium2 NeuronCores (round-robin rewrite).

Decomposition as kernel.py: 729 taps (d,u,v), each a K=64(ci) x M=64(co)
matmul over spatial positions, 4-way concurrent via tile_position
(2 row-groups x 2 col-groups).

Changes vs kernel.py:
 - MM emission is position-ROUND-ROBIN (one sub-MM per position visit)
   instead of 2-3 back-to-back same-position MMs per tap.  Same-position
   MMs serialize (pc-monotone starts + same-subarray), so the old pass-1
   ran ~2-way; round-robin approaches true 4-way.
 - Row clipping: out rows h with h + d*(4-u) < 0 read zero pad rows; the
   MMs now clip them from the AP (rlo per sub) instead of multiplying
   zeros (~9% of all columns).  The 4 seed taps stay unclipped so their
   start=True writes cover the full psum region (they read the memset pad
   rows of tiles 0/1 -- the only memsets left).
 - Class-flip host packing makes the clipping class-independent: h-half-1
   cores get view-relabeled (u,v)->(8-u,8-v), h/w-flipped view images and
   a kernel-flipped weight table (W[:, :, ::-1, ::-1]).  Device program is
   IDENTICAL for all cores; out is unflipped host-side.  (Derivation: with
   Y[ci,u0,v0,r,c] = X[ci,8-u0,8-v0,47-r,47-c] and flipped W-table, the
   class-1 half satisfies out[co,d,47-h',47-w'] = dev_out[co,d,h',w'].)
   This also removes the tc.If(core-id) branch in the input load.
 - PSUM as 8 explicit bank tags (bufs=1) rotated FIFO across walks so a
   new walk lands on the banks whose evacs were emitted earliest.
 - sub-2 walks pair same-sign disparities (-4,-3), (-2,-1), (1,2), (3,4):
   same kidx per tap => one LDW per 2 MMs (dedup'd by _dedup_ldweights).
"""

import os
from collections import deque

import numpy as np
import ml_dtypes

A = 9           # angular resolution
H = 48          # spatial h/w per view
C = 64          # channels (ci = co = 64)
B = 4           # batch
ND = 9          # disparities -4..4
HH = 24         # h rows per core (half)
SUB = 8         # output h rows per psum accumulation group
NSUB = HH // SUB
N_CORES = 8

BF16 = ml_dtypes.bfloat16

POS_ORDER = ((0, 0), (1, 0), (0, 1), (1, 1))


def _geometry():
    """Static tap/tile geometry shared by host packing and device program."""
    pairs = []            # (viewA, viewB-or-None, R)
    # seed tiles first: their 4 views are the full-width (v=4) taps that
    # carry start=True per tile position, so their DMAs must land first.
    pairs.append(((0, 4), (8, 4), HH + 8 * 4))
    pairs.append(((1, 4), (7, 4), HH + 8 * 3))
    for v in range(A):
        for u in range(4):
            if v == 4 and u in (0, 1):
                continue
            pairs.append(((u, v), (8 - u, v), HH + 8 * (4 - u)))
    for k in range(4):
        pairs.append(((4, k), (4, k + 5), HH))
    pairs.append(((4, 4), None, HH))
    # alternate which view of a pair sits on which partition half: row
    # clipping at fixed d hits only u<4 (or only u>4) views, so keeping
    # all u<4 on half 0 would starve one row-group at large |d|.
    pairs = [(vb, va, R) if (j % 2 == 1 and vb is not None) else (va, vb, R)
             for j, (va, vb, R) in enumerate(pairs)]

    view_loc = {}
    offs = []
    off = 0
    for j, (va, vb, R) in enumerate(pairs):
        view_loc[va] = (j, 0)
        if vb is not None:
            view_loc[vb] = (j, 1)
        offs.append(off)
        off += R * H
    F = off

    taps = []  # (u, v, rh) in tile order; ch assigned per walk
    for j, (va, vb, R) in enumerate(pairs):
        for half, view in ((0, va), (1, vb)):
            if view is None:
                continue
            taps.append((view[0], view[1], half))
    assert len(taps) == 81
    assert all(v == 4 for (u, v, _) in taps[:4])
    assert {rh for (_, _, rh) in taps[:4]} == {0, 1}
    seeds = {(u, v) for (u, v, _) in taps[:4]}

    return pairs, view_loc, offs, F, taps, seeds


_PAIRS, _VIEW_LOC, _OFFS, _F, _TAPS, _SEEDS = _geometry()
_NC_CACHE = {}


def _rlo(d, u, v, sub):
    """Rows clipped from the top of this sub's 8-row window (class-0
    geometry; out row h needs x row h + d*(4-u) >= 0).  Seeds unclipped."""
    if (u, v) in _SEEDS:
        return 0
    return max(0, min(SUB, -d * (4 - u) - sub * SUB))


def _walk_cols(tap, dsubs):
    """Exact column count this tap contributes to its quadrant in a walk
    covering dsubs.  The (4,4) singleton is counted per-rh (half-width)."""
    u, v, rh = tap
    c = 0
    for (d, sub) in dsubs:
        if (u, v) == (4, 4):
            c += SUB * (H // 2)
            continue
        r = _rlo(d, u, v, sub)
        if r < SUB:
            c += (SUB - r) * (H - abs(d * (4 - v)))
    return c


def _assign_ch(dsubs):
    """Per-walk greedy column-half assignment balancing quadrant columns
    within each row-half.  Seeds forced to opposite ch so every position's
    queue starts with a full-extent start=True tap.  Returns
    {(u,v): ch} plus ("44", rh) entries for the singleton's half-MMs."""
    load = {p: 0 for p in POS_ORDER}
    chmap = {}
    for rh in (0, 1):
        rtaps = [t for t in _TAPS if t[2] == rh and (t[0], t[1]) != (4, 4)]
        sd = [t for t in rtaps if (t[0], t[1]) in _SEEDS]
        for ch, t in enumerate(sd):
            chmap[(t[0], t[1])] = ch
            load[(rh, ch)] += _walk_cols(t, dsubs)
        rest = sorted([t for t in rtaps if (t[0], t[1]) not in _SEEDS],
                      key=lambda t: -_walk_cols(t, dsubs))
        for t in rest:
            ch = 0 if load[(rh, 0)] <= load[(rh, 1)] else 1
            chmap[(t[0], t[1])] = ch
            load[(rh, ch)] += _walk_cols(t, dsubs)
    # (4,4) singleton: one half-width MM per rh, each on the lighter ch
    c44 = _walk_cols((4, 4, 0), dsubs)
    for rh in (0, 1):
        ch = 0 if load[(rh, 0)] <= load[(rh, 1)] else 1
        chmap[("44", rh)] = ch
        load[(rh, ch)] += c44
    return chmap


def _build_nc(repeat=1):
    import concourse.bacc as bacc
    import concourse.mybir as mybir
    import concourse.tile as tile

    variant = os.environ.get("KVARIANT", "full")  # timing experiments only

    nc = bacc.Bacc(None, target_bir_lowering=False)
    xwin_d = nc.dram_tensor("xwin", [128, _F], mybir.dt.bfloat16,
                            kind="ExternalInput")
    wt_d = nc.dram_tensor("wt", [128, A * A * C], mybir.dt.bfloat16,
                          kind="ExternalInput")
    out_d = nc.dram_tensor("out", [C, ND * NSUB * SUB * H], mybir.dt.float32,
                           kind="ExternalOutput")

    with tile.TileContext(nc) as tc:
        with tc.tile_pool(name="xw", bufs=1) as xpool, \
             tc.tile_pool(name="wp", bufs=1) as wpool, \
             tc.tile_pool(name="ps", bufs=1, space="PSUM") as ppool, \
             tc.tile_pool(name="ob", bufs=4) as opool:

            # resident weight taps; DMA first so the seed MMs aren't blocked
            wtr = wpool.tile([128, A * A * C], mybir.dt.bfloat16, tag="wtr")
            nc.sync.dma_start(out=wtr[:], in_=wt_d[:])

            xtiles = []
            xviews = []
            for j, (va, vb, R) in enumerate(_PAIRS):
                t = xpool.tile([128, R * H], mybir.dt.bfloat16, tag=f"x{j}")
                xtiles.append(t)
                xviews.append(t[:].rearrange("p (r c) -> p r c", r=R, c=H))

            def load_x():
                # Uniform class-0 geometry for every core (class-flip host
                # packing): view block rows [0, 4au) are pad, valid image
                # rows land at [4au, R).  Only the two SEED tiles' pad rows
                # are ever read (other taps are row-clipped), so only those
                # get DVE memsets.
                for j, (va, vb, R) in enumerate(_PAIRS):
                    pad = ((R - HH) // 8) * 4  # 4*au
                    if pad and j < 2:
                        nc.vector.memset(xtiles[j][:, 0:pad * H], 0.0)
                    nc.sync.dma_start(
                        out=xtiles[j][:, pad * H:R * H],
                        in_=xwin_d[:, _OFFS[j] + pad * H:_OFFS[j] + R * H])

            pool_tags = deque([f"pb{i}" for i in range(8)])

            def alloc_tiles(dsubs):
                tiles = {}
                order = []
                for (d, sub) in dsubs:
                    for rh in (0, 1):
                        tag = pool_tags.popleft()
                        tiles[(d, sub, rh)] = ppool.tile(
                            [128, SUB * H], mybir.dt.float32,
                            name=tag, tag=tag)
                        order.append(tag)
                return tiles, order

            def mm(d, sub, tap, ch, tiles, started):
                (u, v, rh) = tap
                j, half = _VIEW_LOC[(u, v)]
                au = abs(4 - u)
                sft = d * (4 - u)
                row0 = sub * SUB + sft + 4 * au
                sv = d * (4 - v)
                wlo = max(0, -sv)
                whi = min(H, H - sv)
                rlo = _rlo(d, u, v, sub)
                if rlo >= SUB:
                    return
                if variant == "halfcols":
                    # timing-only: halve every MM's width (same instruction
                    # count) to separate issue-rate-bound from datapath-bound
                    whi = wlo + max(1, (whi - wlo) // 2)
                elif variant == "fullw2":
                    # timing-only: force full-extent flat 2D APs on both
                    # sides (wrong numerics; +22% columns) to quantify the
                    # cost of clipped/3D access patterns
                    sv = 0
                    wlo, whi, rlo = 0, H, 0
                rhs = xviews[j][rh * 64:(rh + 1) * 64,
                                row0 + rlo:row0 + SUB,
                                wlo + sv:whi + sv]
                kh, kw = (u, v) if d <= 0 else (8 - u, 8 - v)
                kidx = kh * A + kw
                lhsT = wtr[rh * 64:(rh + 1) * 64,
                           kidx * C:(kidx + 1) * C]
                pt = tiles[(d, sub, rh)]
                key = (d, sub, rh, ch)
                if rlo == 0 and wlo == 0 and whi == H:
                    outap = pt[ch * 64:(ch + 1) * 64, :]
                else:
                    ptv = pt[:].rearrange("p (r c) -> p r c", r=SUB, c=H)
                    outap = ptv[ch * 64:(ch + 1) * 64, rlo:SUB, wlo:whi]
                nc.tensor.matmul(
                    outap, lhsT, rhs,
                    start=(key not in started),
                    stop=False,
                    tile_position=(rh * 64, ch * 64),
                    skip_group_check=True,
                )
                started.add(key)

            def mm44(d, sub, chmap, tiles, started):
                # (4,4) singleton: no shift/clip for any d; split into two
                # half-width MMs, one per row-half (data duplicated on both
                # tile halves), each on that rh's lightest quadrant.
                j, _ = _VIEW_LOC[(4, 4)]
                kidx = 4 * A + 4
                row0 = sub * SUB
                for rh, c0, c1 in ((0, 0, H // 2), (1, H // 2, H)):
                    ch = chmap[("44", rh)]
                    rhs = xviews[j][rh * 64:(rh + 1) * 64,
                                    row0:row0 + SUB, c0:c1]
                    lhsT = wtr[rh * 64:(rh + 1) * 64,
                               kidx * C:(kidx + 1) * C]
                    pt = tiles[(d, sub, rh)]
                    ptv = pt[:].rearrange("p (r c) -> p r c", r=SUB, c=H)
                    outap = ptv[ch * 64:(ch + 1) * 64, 0:SUB, c0:c1]
                    key = (d, sub, rh, ch)
                    nc.tensor.matmul(
                        outap, lhsT, rhs,
                        start=(key not in started), stop=False,
                        tile_position=(rh * 64, ch * 64),
                        skip_group_check=True,
                    )
                    started.add(key)

            def emit1(d, sub, tap, chmap, tiles, started):
                if (tap[0], tap[1]) == (4, 4):
                    mm44(d, sub, chmap, tiles, started)
                else:
                    mm(d, sub, tap, chmap[(tap[0], tap[1])], tiles, started)

            def _variant_chmap(chmap):
                # timing-only: force every tap onto ch0 (or ch1) positions
                if variant in ("ch0only", "ch1only"):
                    f = 0 if variant == "ch0only" else 1
                    return {k: f for k in chmap}
                return chmap

            def emit_walk(dsubs, tiles, started):
                # 4-position round-robin, one sub-MM per visit; each tap's
                # dsubs are consecutive in its position queue (same lhsT =>
                # LDWs dedup to one per tap).
                chmap = _variant_chmap(_assign_ch(dsubs))
                posq = {p: [] for p in POS_ORDER}
                for t in _TAPS:
                    key = ("44", 0) if (t[0], t[1]) == (4, 4) else (t[0], t[1])
                    posq[(t[2], chmap[key])].append(t)
                # Duration-sort each queue (seed pinned first): the strict
                # round-robin issue is gated by each round's slowest MM, so
                # rounds should carry like-sized MMs.  E[max of 4 mixed
                # widths] ~ 1.15x mean costs ~15% of the 4-way rate.
                for p in POS_ORDER:
                    sd = [t for t in posq[p] if (t[0], t[1]) in _SEEDS]
                    rest = sorted(
                        [t for t in posq[p] if (t[0], t[1]) not in _SEEDS],
                        key=lambda t: -_walk_cols(t, dsubs))
                    posq[p] = sd + rest
                queues = [[(t, d, s) for t in posq[p] for (d, s) in dsubs]
                          for p in POS_ORDER]
                while any(queues):
                    for q in queues:
                        if q:
                            t, d, s = q.pop(0)
                            emit1(d, s, t, chmap, tiles, started)

            def emit_load_walk(dsubs, tiles, started):
                # Tile-arrival-order emission: interleave consecutive PAIRS
                # of x-tiles so the PE runs ~4-way without waiting on
                # undelivered tiles.  ch is assigned per GROUP so each
                # 2-pair group covers all 4 positions, with the running
                # quadrant load deciding which tap gets which ch.
                groups = [list(range(g, min(g + 2, len(_PAIRS))))
                          for g in range(0, len(_PAIRS), 2)]
                ti = 0
                tap_of_pair = []
                for j, (va, vb, R) in enumerate(_PAIRS):
                    n = 1 if vb is None else 2
                    tap_of_pair.append(_TAPS[ti:ti + n])
                    ti += n
                load = {p: 0 for p in POS_ORDER}
                chmap = {}
                for grp in groups:
                    gtaps = [t for j in grp for t in tap_of_pair[j]]
                    for rh in (0, 1):
                        rtaps = sorted(
                            [t for t in gtaps
                             if t[2] == rh and (t[0], t[1]) != (4, 4)],
                            key=lambda t: -_walk_cols(t, dsubs))
                        for t in rtaps:
                            ch = 0 if load[(rh, 0)] <= load[(rh, 1)] else 1
                            if (t[0], t[1]) in chmap:
                                ch = chmap[(t[0], t[1])]
                            else:
                                chmap[(t[0], t[1])] = ch
                            load[(rh, ch)] += _walk_cols(t, dsubs)
                            # force the group's 2nd same-rh tap to the
                            # other ch so the group covers 4 positions
                            load[(rh, 1 - ch)] += 0
                        if len(rtaps) == 2:
                            a, b = rtaps
                            if chmap[(a[0], a[1])] == chmap[(b[0], b[1])]:
                                old = chmap[(b[0], b[1])]
                                chmap[(b[0], b[1])] = 1 - old
                                load[(rh, old)] -= _walk_cols(b, dsubs)
                                load[(rh, 1 - old)] += _walk_cols(b, dsubs)
                    for rh in (0, 1):
                        if ("44", rh) not in chmap and any(
                                (t[0], t[1]) == (4, 4) for t in gtaps):
                            chmap[("44", rh)] = (
                                0 if load[(rh, 0)] <= load[(rh, 1)] else 1)
                    chmap = _variant_chmap(chmap)
                    for (d, s) in dsubs:
                        for t in gtaps:
                            emit1(d, s, t, chmap, tiles, started)

            def evac(di, sub, ptA, ptB):
                ot = opool.tile([64, SUB * H], mybir.dt.float32, tag="ot")
                # Act engine does the copy; DVE the 3 accumulating adds.
                # Cross-partition reads only from PSUM, one PSUM operand/op.
                nc.scalar.activation(ot[:], ptA[0:64, :],
                                     mybir.ActivationFunctionType.Copy)
                if variant == "tinyadd":
                    # timing-only: same dep structure, 1/48th the DVE data
                    nc.vector.tensor_add(ot[:, 0:8], ot[:, 0:8],
                                         ptA[64:128, 0:8])
                    nc.vector.tensor_add(ot[:, 0:8], ot[:, 0:8],
                                         ptB[0:64, 0:8])
                    nc.vector.tensor_add(ot[:, 0:8], ot[:, 0:8],
                                         ptB[64:128, 0:8])
                else:
                    nc.vector.tensor_add(ot[:], ot[:], ptA[64:128, :])
                    nc.vector.tensor_add(ot[:], ot[:], ptB[0:64, :])
                    nc.vector.tensor_add(ot[:], ot[:], ptB[64:128, :])
                seg = (di * NSUB + sub) * SUB * H
                nc.sync.dma_start(out=out_d[:, seg:seg + SUB * H], in_=ot[:])

            def emit_evacs(dsubs, tiles, order):
                for (d, sub) in dsubs:
                    evac(d + 4, sub, tiles[(d, sub, 0)], tiles[(d, sub, 1)])
                pool_tags.extend(order)

            def emit_sweep():
                started = set()
                # load walk: d=-4,-3 subs 0,1 (8 banks), overlapping the
                # HBM-bound input stream in tile-arrival order
                ds_load = [(-4, 0), (-4, 1), (-3, 0), (-3, 1)]
                tl, ol = alloc_tiles(ds_load)
                emit_load_walk(ds_load, tl, started)
                emit_evacs(ds_load, tl, ol)
                # paired sub-2 walk for d=-4,-3 (same kidx => shared LDW)
                ds = [(-4, 2), (-3, 2)]
                tb, ob = alloc_tiles(ds)
                emit_walk(ds, tb, started)
                emit_evacs(ds, tb, ob)
                for dpair in ((-2, -1), (0,), (1, 2), (3, 4)):
                    for d in dpair:
                        ds = [(d, 0), (d, 1)]
                        t_, o_ = alloc_tiles(ds)
                        emit_walk(ds, t_, started)
                        emit_evacs(ds, t_, o_)
                    ds = [(d, 2) for d in dpair]
                    t_, o_ = alloc_tiles(ds)
                    emit_walk(ds, t_, started)
                    emit_evacs(ds, t_, o_)

            if repeat == 1:
                load_x()
                emit_sweep()
            elif variant == "loopall":
                # timing: input DMA repeats with the sweep => loop slope
                # approximates a full single-shot exec
                with tc.For_i(0, repeat, 1):
                    load_x()
                    emit_sweep()
            else:
                # timing: repeat the compute sweep in a hardware loop
                load_x()
                with tc.For_i(0, repeat, 1):
                    emit_sweep()

    _dedup_ldweights(nc, drop_all=(variant == "noldw"))
    if variant != "nocollapse":
        _collapse_pe_sem_incs(nc)
    nc.finalize()
    return nc


def _collapse_pe_sem_incs(nc):
    """Collapse per-MM semaphore increments into one bulk sem-inc at the
    end of each run of WAIT-FREE PE instructions.  Per-MM then_incs
    serialize on the EVT_SEM register (~26ns each, tensor-engine doc) --
    one inc per MM caps the sweep at ~26ns/MM regardless of tile-position
    concurrency.  Moving incs LATER is always data-safe (thresholds clear
    later, never earlier); deadlock is impossible because a run contains
    only wait-free PE instructions, so the PE always reaches the run end
    where the bulk inc fires.  Runs break at any PE instruction that
    waits (walk-first MMs, psum-WAR guards, LDW-vs-MM guards) and at any
    PE instruction that isn't a plain matmul/ldweights."""
    removed = 0
    for fn in nc.m.functions:
        for bb in fn.blocks:
            cur = []  # (instruction, update) collected since last break

            def flush(run):
                nonlocal removed
                by_sem = {}
                for ins, upd in run:
                    by_sem.setdefault(upd.id, []).append((ins, upd))
                for sem, lst in by_sem.items():
                    if len(lst) < 2:
                        continue
                    total = sum(u.update_value for _, u in lst)
                    # 'sem-inc' adds 1 regardless of update_value; the bulk
                    # increment needs the add-immediate form (same mode the
                    # SWDGE DMA completion updates use).
                    lst[-1][1].update_mode = "sem-add-imm"
                    lst[-1][1].update_value = total
                    for ins, upd in lst[:-1]:
                        ins.sync_info.on_update.remove(upd)
                        removed += 1

            for ins in bb.instructions:
                if not str(ins.engine).endswith("PE"):
                    continue
                tn = type(ins).__name__
                si = ins.sync_info
                has_wait = si is not None and bool(si.on_wait)
                if tn not in ("InstMatmult", "InstLdweights"):
                    flush(cur)
                    cur = []
                    continue
                if has_wait:
                    flush(cur)
                    cur = []
                if si and si.on_update:
                    for upd in list(si.on_update):
                        if (upd.sync_type == "semaphore"
                                and upd.update_mode == "sem-inc"):
                            cur.append((ins, upd))
            flush(cur)
    if removed:
        import logging
        logging.getLogger(__name__).info(
            "collapse_pe_sem_incs: removed %d increments", removed)


def _dedup_ldweights(nc, drop_all=False):
    """Remove InstLdweights that reload the stationary operand already
    resident at the same tile position.  The PE keeps independent
    stationary sets per (row, col) tile group, and only an LDW targeting
    the same position clobbers one.  drop_all (timing-only variant):
    remove every sync-free LDW after the first per position."""
    removed = kept = 0
    for bb in nc.m.functions[0].blocks:
        last = {}
        to_remove = []
        for ins in bb.instructions:
            if not str(ins.engine).endswith("PE"):
                continue
            tn = type(ins).__name__
            if tn == "InstLdweights":
                si = ins.sync_info
                has_sync = si is not None and (si.on_wait or si.on_update)
                sig = (str(ins.ins[0]), str(getattr(ins, "tile_position", None)),
                       str(getattr(ins, "perf_mode", None)))
                pos = str(getattr(ins, "tile_position", None))
                if drop_all:
                    sig = True
                if not has_sync and last.get(pos) == sig:
                    to_remove.append(ins)
                    removed += 1
                else:
                    last[pos] = sig
                    kept += 1
            elif tn == "InstMatmult":
                continue
            else:
                last.clear()
        for ins in to_remove:
            bb.instructions.remove(ins)
    if removed:
        import logging
        logging.getLogger(__name__).info(
            "dedup_ldweights: removed %d, kept %d", removed, kept)


def get_nc(repeat=1):
    key = ("nc", repeat, os.environ.get("KVARIANT", "full"))
    if key not in _NC_CACHE:
        _NC_CACHE[key] = _build_nc(repeat)
    return _NC_CACHE[key]


def prepare_inputs(x, W):
    """Host-side packing: per-core xwin [128,F] bf16 + per-class weights.

    h-half-1 cores get the class-flip: view (u0,v0) holds
    X[8-u0, 8-v0, ::-1(h), ::-1(w)] and the weight table is kernel-flipped,
    which makes the device program identical to the h-half-0 one."""
    x = np.asarray(x, dtype=np.float32)
    W = np.asarray(W, dtype=np.float32)
    # X5[b,u,v,ci,h,w]
    X5 = np.ascontiguousarray(
        x.reshape(B, C, H, A, H, A).transpose(0, 3, 5, 1, 2, 4)
    ).astype(BF16)

    xwins = []
    for core in range(N_CORES):
        b, hh = divmod(core, 2)
        V = X5[b] if hh == 0 else X5[b, ::-1, ::-1, :, ::-1, ::-1]
        xw = np.zeros((128, _F), dtype=BF16)
        for j, (va, vb, R) in enumerate(_PAIRS):
            # the (4,4) singleton is duplicated onto the (otherwise empty)
            # second half of its tile so its matmul can be split across
            # both row-halves for quadrant load balance.
            for half, view in ((0, va), (1, vb if vb is not None else va)):
                u, v = view
                lo = -4 * abs(4 - u)
                ve = min(H, lo + R)
                blk = V[u, v, :, 0:ve, :]  # [64, ve, 48]
                dst = xw[half * 64:(half + 1) * 64,
                         _OFFS[j]:_OFFS[j] + R * H].reshape(64, R, H)
                dst[:, -lo:ve - lo, :] = blk
        xwins.append(xw)

    # wt[ci + 64*half, (kh*9+kw)*64 + co] = Wc[co, ci, kh, kw], where Wc is
    # W for h-half-0 cores and W[:, :, ::-1, ::-1] for h-half-1 cores.
    def pack_w(Wc):
        w1 = np.ascontiguousarray(
            Wc.transpose(1, 2, 3, 0).reshape(C, A * A * C)).astype(BF16)
        return np.concatenate([w1, w1], axis=0)

    return xwins, (pack_w(W), pack_w(W[:, :, ::-1, ::-1]))


def assemble_output(results):
    """results: list of 8 dicts with 'out' [64, ND*NSUB*SUB*H] fp32."""
    full = np.empty((B, C, ND, H, H), dtype=np.float32)
    for core in range(N_CORES):
        b, hh = divmod(core, 2)
        oc = np.asarray(results[core]["out"]).reshape(C, ND, HH, H)
        if hh == 0:
            full[b, :, :, 0:HH, :] = oc
        else:
            full[b, :, :, HH:H, :] = oc[:, :, ::-1, ::-1]
    return full


def make_in_maps(x, W):
    xwins, (wt0, wt1) = prepare_inputs(x, W)
    return [{"xwin": xwins[c], "wt": (wt0 if c % 2 == 0 else wt1)}
            for c in range(N_CORES)]


def kernel(x, W):
    from concourse.bass_utils import run_bass_kernel_spmd

    nc = get_nc()
    in_maps = make_in_maps(x, W)
    res = run_bass_kernel_spmd(nc, in_maps, core_ids=list(range(N_CORES)))
    return assemble_output(res.results)
